# revision 1
# baseline (speedup 1.0000x reference)
"""2-layer GATv2 over 50k nodes / 1.6M edges on 8 trn2 NeuronCores.

Strategy (self-contained; shapes hardcoded for this problem):
  - Node-parallel dst sharding: nodes are degree-sorted and dealt round-robin
    to 8 cores (balanced slot counts); each core owns 6272 dst nodes.
  - Per dst node, incoming edges live in up to D=64 "slots" (max degree 61);
    per-128-node-tile slot count Dt comes from the degree sort, cutting padded
    work from 64 to ~avg-degree slots.
  - att is folded into the weights on the host (u = att*z); leaky-relu logits
    are computed as sum_pos lrelu(u) - sum_neg lrelu(-u) via ACT with a host
    sign-permutation of feature columns; messages are recovered from u via a
    1/att columnwise multiply (exact up to fp rounding).
  - Gather of xl rows via indirect DMA (gpsimd SWDGE); pad slots masked by
    -1e30 logit offsets.
  - Layer-1 output h is transposed on PE, AllGathered across cores, and used
    as lhsT for layer-2 GEMMs.
"""
import os
os.environ.setdefault("JAX_PLATFORMS", "cpu")
import sys
if "/opt/trn_rl_repo" not in sys.path:
    sys.path.insert(0, "/opt/trn_rl_repo")
import numpy as np
import concourse.bass as bass
import concourse.bacc as bacc
import concourse.mybir as mybir
import concourse.tile as tile
from concourse import bass_utils
from concourse.masks import make_identity

f32 = mybir.dt.float32
i32 = mybir.dt.int32
AX = mybir.AxisListType
OP = mybir.AluOpType
AF = mybir.ActivationFunctionType

N = 50000
NCORES = 8
NP = 50176          # 8 * 6272, multiple of 1024
SH = NP // NCORES   # 6272 = 49 * 128
TPS = SH // 128     # 49 tiles per shard
F_IN = 256
H = 128
C = 64
DMAX = 64
NEG = 0.2
EPS = 1e-16

GATHER_MODE = "per_d"   # "per_d" | "multi"
CCE_ADD = True          # add xr during the gather DMA (CCE accumulate)
DEBUG_TAPS = False      # expose intermediate DRAM tensors as outputs

LAST_RESULT = None
_PROGRAM_CACHE = {}


def ts(i, s):
    return slice(i * s, (i + 1) * s)


def ceil4(v):
    return max(4, (int(v) + 3) // 4 * 4)


def build_program(Dts, Fp1, Fp2):
    key = (tuple(Dts), Fp1, Fp2, GATHER_MODE, CCE_ADD, DEBUG_TAPS)
    if key in _PROGRAM_CACHE:
        return _PROGRAM_CACHE[key]
    nc = bacc.Bacc("TRN2", target_bir_lowering=False, debug=False,
                   enable_asserts=False, num_devices=NCORES)

    xT = nc.dram_tensor("xT", [F_IN, NP], f32, kind="ExternalInput")
    xTo = nc.dram_tensor("xTo", [F_IN, SH], f32, kind="ExternalInput")
    wl1 = nc.dram_tensor("wl1", [F_IN, H + 1], f32, kind="ExternalInput")
    wr1 = nc.dram_tensor("wr1", [F_IN, H + 1], f32, kind="ExternalInput")
    wl2 = nc.dram_tensor("wl2", [H, C + 1], f32, kind="ExternalInput")
    wr2 = nc.dram_tensor("wr2", [H, C + 1], f32, kind="ExternalInput")
    slot = nc.dram_tensor("slot", [SH, DMAX], i32, kind="ExternalInput")
    offs = nc.dram_tensor("offs", [SH, DMAX], f32, kind="ExternalInput")
    rc1 = nc.dram_tensor("rc1", [128, H], f32, kind="ExternalInput")
    cb1 = nc.dram_tensor("cb1", [128, H], f32, kind="ExternalInput")
    rc2 = nc.dram_tensor("rc2", [128, C], f32, kind="ExternalInput")
    cb2 = nc.dram_tensor("cb2", [128, C], f32, kind="ExternalInput")
    outc = nc.dram_tensor("outc", [SH, C], f32, kind="ExternalOutput")

    xl1t = nc.dram_tensor("xl1t", [NP, H + 1], f32, kind="Internal")
    xr1t = nc.dram_tensor("xr1t", [SH, H + 1], f32, kind="Internal")
    h1T = nc.dram_tensor("h1T", [H, SH], f32, kind="Internal")
    h1Tf = nc.dram_tensor("h1Tf", [NCORES * H, SH], f32, kind="Internal",
                          addr_space="Shared")
    xl2t = nc.dram_tensor("xl2t", [NP, C + 1], f32, kind="Internal")
    xr2t = nc.dram_tensor("xr2t", [SH, C + 1], f32, kind="Internal")

    taps = {}
    if DEBUG_TAPS:
        taps["t_xl1"] = nc.dram_tensor("t_xl1", [NP, H + 1], f32,
                                       kind="ExternalOutput")
        taps["t_xr1"] = nc.dram_tensor("t_xr1", [SH, H + 1], f32,
                                       kind="ExternalOutput")
        taps["t_h1T"] = nc.dram_tensor("t_h1T", [H, SH], f32,
                                       kind="ExternalOutput")
        taps["t_xl2"] = nc.dram_tensor("t_xl2", [NP, C + 1], f32,
                                       kind="ExternalOutput")

    with tile.TileContext(nc) as tc:
        # ---------------- Phase A: layer-1 GEMMs ----------------
        with (
            tc.tile_pool(name="paw", bufs=1) as pw,
            tc.tile_pool(name="pa", bufs=4) as pa,
            tc.tile_pool(name="pap", bufs=4, space="PSUM") as pp,
        ):
            wl_t = pw.tile([128, 2, H + 1], f32)
            wr_t = pw.tile([128, 2, H + 1], f32)
            for k in range(2):
                nc.sync.dma_start(out=wl_t[:, k, :], in_=wl1.ap()[ts(k, 128), :])
                nc.sync.dma_start(out=wr_t[:, k, :], in_=wr1.ap()[ts(k, 128), :])
            for t in range(NP // 128):
                xt_t = pa.tile([128, 2, 128], f32, tag="xt")
                for k in range(2):
                    nc.sync.dma_start(out=xt_t[:, k, :],
                                      in_=xT.ap()[ts(k, 128), ts(t, 128)])
                ps_t = pp.tile([128, H + 1], f32, tag="ps")
                for k in range(2):
                    nc.tensor.matmul(out=ps_t[:], lhsT=xt_t[:, k, :],
                                     rhs=wl_t[:, k, :],
                                     start=(k == 0), stop=(k == 1))
                o_t = pa.tile([128, H + 1], f32, tag="o")
                nc.scalar.copy(out=o_t[:], in_=ps_t[:])
                nc.sync.dma_start(out=xl1t.ap()[ts(t, 128), :], in_=o_t[:])
            for t in range(TPS):
                xt_t = pa.tile([128, 2, 128], f32, tag="xt")
                for k in range(2):
                    nc.sync.dma_start(out=xt_t[:, k, :],
                                      in_=xTo.ap()[ts(k, 128), ts(t, 128)])
                ps_t = pp.tile([128, H + 1], f32, tag="ps")
                for k in range(2):
                    nc.tensor.matmul(out=ps_t[:], lhsT=xt_t[:, k, :],
                                     rhs=wr_t[:, k, :],
                                     start=(k == 0), stop=(k == 1))
                o_t = pa.tile([128, H + 1], f32, tag="o")
                nc.scalar.copy(out=o_t[:], in_=ps_t[:])
                nc.sync.dma_start(out=xr1t.ap()[ts(t, 128), :], in_=o_t[:])

        # ---------------- Phase B: layer-1 edge phase ----------------
        edge_phase(nc, tc, Dts, Fp1, H, xl1t, xr1t, slot, offs,
                   rc1, cb1, relu=True, out_dram=None, hT_dram=h1T)

        # ---------------- Phase C: AllGather h1T ----------------
        nc.gpsimd.collective_compute(
            "AllGather", OP.bypass,
            replica_groups=[list(range(NCORES))],
            ins=[h1T.ap()], outs=[h1Tf.ap()])

        # ---------------- Phase D: layer-2 GEMMs ----------------
        with (
            tc.tile_pool(name="pdw", bufs=1) as pw2,
            tc.tile_pool(name="pd", bufs=4) as pd,
            tc.tile_pool(name="pdp", bufs=4, space="PSUM") as pp2,
        ):
            wl2_t = pw2.tile([128, C + 1], f32)
            nc.sync.dma_start(out=wl2_t[:], in_=wl2.ap())
            wr2_t = pw2.tile([128, C + 1], f32)
            nc.sync.dma_start(out=wr2_t[:], in_=wr2.ap())
            for t in range(NP // 128):
                r, cc = t // TPS, t % TPS
                ht_t = pd.tile([128, 128], f32, tag="ht")
                nc.sync.dma_start(out=ht_t[:],
                                  in_=h1Tf.ap()[ts(r, 128), ts(cc, 128)])
                ps2_t = pp2.tile([128, C + 1], f32, tag="ps2")
                nc.tensor.matmul(out=ps2_t[:], lhsT=ht_t[:], rhs=wl2_t[:],
                                 start=True, stop=True)
                o2_t = pd.tile([128, C + 1], f32, tag="o2")
                nc.scalar.copy(out=o2_t[:], in_=ps2_t[:])
                nc.sync.dma_start(out=xl2t.ap()[ts(t, 128), :], in_=o2_t[:])
            for t in range(TPS):
                ht_t = pd.tile([128, 128], f32, tag="ht")
                nc.sync.dma_start(out=ht_t[:], in_=h1T.ap()[:, ts(t, 128)])
                ps2_t = pp2.tile([128, C + 1], f32, tag="ps2")
                nc.tensor.matmul(out=ps2_t[:], lhsT=ht_t[:], rhs=wr2_t[:],
                                 start=True, stop=True)
                o2_t = pd.tile([128, C + 1], f32, tag="o2")
                nc.scalar.copy(out=o2_t[:], in_=ps2_t[:])
                nc.sync.dma_start(out=xr2t.ap()[ts(t, 128), :], in_=o2_t[:])

        # ---------------- Phase E: layer-2 edge phase ----------------
        edge_phase(nc, tc, Dts, Fp2, C, xl2t, xr2t, slot, offs,
                   rc2, cb2, relu=False, out_dram=outc, hT_dram=None)

        if DEBUG_TAPS:
            with tc.tile_pool(name="ptap", bufs=2) as pt_:
                def dump(dst, src, rows, cols):
                    for t in range(rows // 128):
                        tt = pt_.tile([128, cols], f32, tag="tap")
                        nc.sync.dma_start(out=tt[:],
                                          in_=src.ap()[ts(t, 128), :])
                        nc.sync.dma_start(out=dst.ap()[ts(t, 128), :],
                                          in_=tt[:])
                dump(taps["t_xl1"], xl1t, NP, H + 1)
                dump(taps["t_xr1"], xr1t, SH, H + 1)
                dump(taps["t_h1T"], h1T, H, SH)
                dump(taps["t_xl2"], xl2t, NP, C + 1)

    nc.compile()
    _PROGRAM_CACHE[key] = nc
    return nc


def edge_phase(nc, tc, Dts, Fp, F, xl_tab, xr_tab, slot, offs, rc, cb,
               relu, out_dram, hT_dram):
    with (
        tc.tile_pool(name=f"pz{F}", bufs=2) as pz,
        tc.tile_pool(name=f"pw{F}", bufs=2) as pwv,
        tc.tile_pool(name=f"pb{F}", bufs=2) as pb,
        tc.tile_pool(name=f"pm{F}", bufs=3) as psm,
        tc.tile_pool(name=f"pc{F}", bufs=1) as pcst,
        tc.tile_pool(name=f"po{F}", bufs=2) as pout,
        tc.tile_pool(name=f"pp{F}", bufs=2, space="PSUM") as pps,
    ):
        if relu:
            ident = pcst.tile([128, 128], f32)
            make_identity(nc, ident[:])
            hT_sb = pcst.tile([128, SH], f32)
        rc_t = pcst.tile([128, F], f32)
        nc.sync.dma_start(out=rc_t[:], in_=rc.ap())
        cb_t = pcst.tile([128, F], f32)
        nc.sync.dma_start(out=cb_t[:], in_=cb.ap())

        for t in range(TPS):
            Dt = Dts[t]
            idx_t = pb.tile([128, Dt], i32, tag="idx")
            nc.sync.dma_start(out=idx_t[:], in_=slot.ap()[ts(t, 128), 0:Dt])
            off_t = pb.tile([128, Dt], f32, tag="off")
            nc.sync.dma_start(out=off_t[:], in_=offs.ap()[ts(t, 128), 0:Dt])
            TW = F + 1   # table width: F features + q (= row-sum) column
            xr_t = pb.tile([128, TW], f32, tag="xr")
            nc.sync.dma_start(out=xr_t[:], in_=xr_tab.ap()[ts(t, 128), :])

            # z_t holds: CCE mode -> xr + g (prefill + gather-accumulate);
            #            else    -> g (gather overwrite; pads point to row 0)
            z_t = pz.tile([128, Dt, TW], f32, tag="z")
            if CCE_ADD:
                nc.gpsimd.tensor_copy(
                    out=z_t[:], in_=xr_t[:, None, :].to_broadcast([128, Dt, TW]))
                cop = OP.add
            else:
                cop = OP.bypass
            if GATHER_MODE == "multi":
                nc.gpsimd.indirect_dma_start(
                    out=z_t[:], out_offset=None, in_=xl_tab.ap(),
                    in_offset=bass.IndirectOffsetOnAxis(ap=idx_t[:], axis=0),
                    bounds_check=NP - 1, oob_is_err=False, compute_op=cop)
            else:
                for d in range(Dt):
                    nc.gpsimd.indirect_dma_start(
                        out=z_t[:, d, :], out_offset=None, in_=xl_tab.ap(),
                        in_offset=bass.IndirectOffsetOnAxis(
                            ap=idx_t[:, d:d + 1], axis=0),
                        bounds_check=NP - 1, oob_is_err=False, compute_op=cop)

            # logits: e = 0.8*(sum_pos relu(u) - sum_neg relu(-u))
            #           + 0.2*sigma + offs, with sigma = sum_all u carried in
            #           the q column (z[:, :, F]).
            w_t = pwv.tile([128, Dt, F], f32, tag="w")
            if CCE_ADD:
                lg_t = z_t  # z already includes xr
            else:
                raise NotImplementedError("non-CCE path needs q-col handling")
            nc.scalar.activation(out=w_t[:, :, 0:Fp], in_=lg_t[:, :, 0:Fp],
                                 func=AF.Relu)
            nc.scalar.activation(out=w_t[:, :, Fp:F], in_=lg_t[:, :, Fp:F],
                                 func=AF.Relu, scale=-1.0)
            ep_t = psm.tile([128, Dt], f32, tag="ep")
            nc.vector.tensor_reduce(out=ep_t[:], in_=w_t[:, :, 0:Fp],
                                    axis=AX.X, op=OP.add)
            en_t = psm.tile([128, Dt], f32, tag="en")
            nc.vector.tensor_reduce(out=en_t[:], in_=w_t[:, :, Fp:F],
                                    axis=AX.X, op=OP.add)
            e_t = psm.tile([128, Dt], f32, tag="e")
            nc.vector.scalar_tensor_tensor(out=e_t[:], in0=en_t[:],
                                           scalar=-1.0, in1=ep_t[:],
                                           op0=OP.mult, op1=OP.add)
            # e = 0.8*e0 + offs, then += 0.2*sigma
            nc.vector.scalar_tensor_tensor(out=e_t[:], in0=e_t[:],
                                           scalar=0.8, in1=off_t[:],
                                           op0=OP.mult, op1=OP.add)
            nc.vector.scalar_tensor_tensor(out=e_t[:], in0=z_t[:, :, F],
                                           scalar=0.2, in1=e_t[:],
                                           op0=OP.mult, op1=OP.add)
            mneg_t = psm.tile([128, 1], f32, tag="mneg")
            nc.vector.tensor_reduce(out=mneg_t[:], in_=e_t[:], axis=AX.X,
                                    op=OP.max, negate=True)
            nc.vector.tensor_scalar_min(mneg_t[:], mneg_t[:], 1e29)
            a_t = psm.tile([128, Dt], f32, tag="a")
            nc.scalar.activation(out=a_t[:], in_=e_t[:], func=AF.Exp,
                                 bias=mneg_t[:, :1])
            s_t = psm.tile([128, 1], f32, tag="s")
            nc.vector.tensor_reduce(out=s_t[:], in_=a_t[:], axis=AX.X,
                                    op=OP.add)
            nc.vector.tensor_scalar_add(s_t[:], s_t[:], EPS)
            r_t = psm.tile([128, 1], f32, tag="r")
            nc.vector.reciprocal(out=r_t[:], in_=s_t[:])
            al_t = psm.tile([128, Dt], f32, tag="al")
            nc.vector.tensor_scalar_mul(al_t[:], a_t[:], r_t[:, :1])

            # message aggregation: msg = sum_d alpha_d * g_d. In CCE mode z
            # holds xr+g, and sum_d alpha_d (xr+g) - (sum alpha) xr = sum
            # alpha g, so subtract sa*xr afterwards.
            acc_t = pout.tile([128, F], f32, tag="acc")
            nc.vector.tensor_scalar(out=acc_t[:], in0=z_t[:, 0, 0:F],
                                    scalar1=al_t[:, 0:1], scalar2=None,
                                    op0=OP.mult)
            for d in range(1, Dt):
                nc.vector.scalar_tensor_tensor(
                    out=acc_t[:], in0=z_t[:, d, 0:F], scalar=al_t[:, d:d + 1],
                    in1=acc_t[:], op0=OP.mult, op1=OP.add)
            hh_t = pout.tile([128, F], f32, tag="hh")
            saneg_t = psm.tile([128, 1], f32, tag="saneg")
            nc.vector.tensor_reduce(out=saneg_t[:], in_=al_t[:],
                                    axis=AX.X, op=OP.add, negate=True)
            nc.vector.scalar_tensor_tensor(
                out=hh_t[:], in0=xr_t[:, 0:F], scalar=saneg_t[:, :1],
                in1=acc_t[:], op0=OP.mult, op1=OP.add)
            nc.vector.tensor_tensor(out=hh_t[:], in0=hh_t[:],
                                    in1=rc_t[:], op=OP.mult)
            nc.vector.tensor_tensor(out=hh_t[:], in0=hh_t[:], in1=cb_t[:],
                                    op=OP.add)
            if relu:
                nc.vector.tensor_scalar_max(hh_t[:], hh_t[:], 0.0)
                pt_t = pps.tile([128, 128], f32, tag="pt")
                nc.tensor.transpose(out=pt_t[:], in_=hh_t[:],
                                    identity=ident[:])
                nc.scalar.copy(out=hT_sb[:, ts(t, 128)], in_=pt_t[:])
            else:
                nc.sync.dma_start(out=out_dram.ap()[ts(t, 128), :],
                                  in_=hh_t[:])
        if relu:
            nc.sync.dma_start(out=hT_dram.ap(), in_=hT_sb[:])


def prepare_host(x, edge_index, Wl1, Wr1, att1, b1, Wl2, Wr2, att2, b2):
    src = np.asarray(edge_index[0], dtype=np.int64)
    dst = np.asarray(edge_index[1], dtype=np.int64)
    x = np.asarray(x, dtype=np.float32)

    deg = np.bincount(dst, minlength=NP).astype(np.int64)
    assert deg.max() <= DMAX, f"max degree {deg.max()} > {DMAX}"
    order = np.argsort(-deg, kind="stable")
    q = np.arange(NP)
    new_of = np.empty(NP, dtype=np.int64)
    new_of[order] = (q % NCORES) * SH + q // NCORES
    glob_of_new = np.empty(NP, dtype=np.int64)
    glob_of_new[new_of] = np.arange(NP)

    # slot tables (values are NEW ids; rows ordered by NEW id)
    eorder = np.argsort(dst, kind="stable")
    s_src = src[eorder]
    s_dst = dst[eorder]
    starts = np.zeros(NP, dtype=np.int64)
    starts[1:] = np.cumsum(deg)[:-1]
    pos = np.arange(len(s_dst)) - starts[s_dst]
    # CCE mode skips pads via bounds check (idx=NP); overwrite mode points
    # pads at row 0 (finite garbage, masked by offs / alpha=0 downstream)
    pad_idx = NP if CCE_ADD else 0
    slot_g = np.full((NP, DMAX), pad_idx, dtype=np.int32)
    offs_g = np.full((NP, DMAX), -1e30, dtype=np.float32)
    slot_g[s_dst, pos] = new_of[s_src].astype(np.int32)
    offs_g[s_dst, pos] = 0.0
    slot_new = slot_g[glob_of_new]
    offs_new = offs_g[glob_of_new]

    deg_sorted = deg[order]
    Dts = tuple(ceil4(max(deg_sorted[1024 * t], 1)) for t in range(TPS))

    att1 = np.asarray(att1, np.float32)
    att2 = np.asarray(att2, np.float32)
    assert np.abs(att1).min() > 1e-8 and np.abs(att2).min() > 1e-8
    p1 = np.argsort(att1 < 0, kind="stable")
    Fp1 = int((att1 >= 0).sum())
    p2 = np.argsort(att2 < 0, kind="stable")
    Fp2 = int((att2 >= 0).sum())
    # fold att into weight columns, sign-permute, and append a row-sum
    # column (the q/sigma channel: sum_f u = x @ wsum)
    def fold(W, att, perm, rowperm=None):
        Wa = (np.asarray(W, np.float32) * att)
        if rowperm is not None:
            Wa = Wa[rowperm, :]
        Wp = Wa[:, perm]
        return np.ascontiguousarray(
            np.concatenate([Wp, Wp.sum(1, keepdims=True)], axis=1), np.float32)

    Wl1a = fold(Wl1, att1, p1)
    Wr1a = fold(Wr1, att1, p1)
    Wl2a = fold(Wl2, att2, p2, rowperm=p1)
    Wr2a = fold(Wr2, att2, p2, rowperm=p1)
    rc1_row = (1.0 / att1[p1]).astype(np.float32)
    rc2_row = (1.0 / att2[p2]).astype(np.float32)
    b1_row = np.asarray(b1, np.float32)[p1]
    b2_row = np.asarray(b2, np.float32)[p2]

    xp = np.zeros((NP, F_IN), np.float32)
    xp[:N] = x
    xT_perm = np.ascontiguousarray(xp[glob_of_new].T)

    rep = lambda row: np.ascontiguousarray(np.tile(row[None, :], (128, 1)))
    common = dict(
        xT=xT_perm, wl1=Wl1a, wr1=Wr1a, wl2=Wl2a, wr2=Wr2a,
        rc1=rep(rc1_row), cb1=rep(b1_row), rc2=rep(rc2_row), cb2=rep(b2_row))
    in_maps = []
    for c in range(NCORES):
        m = dict(common)
        m["xTo"] = np.ascontiguousarray(xT_perm[:, ts(c, SH)])
        m["slot"] = np.ascontiguousarray(slot_new[ts(c, SH)])
        m["offs"] = np.ascontiguousarray(offs_new[ts(c, SH)])
        in_maps.append(m)
    return in_maps, Dts, Fp1, Fp2, glob_of_new, p2


def kernel(**inputs):
    global LAST_RESULT, LAST_RUN_WALL
    import time as _time
    in_maps, Dts, Fp1, Fp2, glob_of_new, p2 = prepare_host(**inputs)
    nc = build_program(Dts, Fp1, Fp2)
    _t0 = _time.time()
    res = bass_utils.run_bass_kernel_spmd(nc, in_maps,
                                          core_ids=list(range(NCORES)))
    LAST_RUN_WALL = _time.time() - _t0
    LAST_RESULT = res
    out_new = np.concatenate([res.results[c]["outc"] for c in range(NCORES)],
                             axis=0)
    out_glob = np.empty((NP, C), np.float32)
    out_glob[glob_of_new] = out_new
    return np.ascontiguousarray(out_glob[:N][:, np.argsort(p2)])



# revision 3
# speedup vs baseline: 4.4679x; 4.4679x over previous
"""2-layer GATv2 over 50k nodes / 1.6M edges on 8 trn2 NeuronCores.

Strategy (self-contained; shapes hardcoded for this problem):
  - Node-parallel dst sharding: nodes are degree-sorted and dealt round-robin
    to 8 cores (balanced slot counts); each core owns 6272 dst nodes.
  - Host->device traffic is minimized (the axon tunnel is ~30-70 MB/s):
    each core receives only its OWN x shard (fp16, [256, 6272]); the full
    xl tables needed for the src gathers are built on-device: each core
    computes xl for its shard and an AllGather forms the [50176, F] table.
  - Per dst node, incoming edges live in up to D=64 "slots" (max degree 61);
    per-128-node-tile slot count Dt comes from the degree sort; slot tables
    are column-packed to sum(Dt) on the host. Pad masks are built on-device
    from per-node degrees (iota >= deg -> -1e30).
  - att is folded into the weights on the host (u = att*z) with a sign
    permutation of feature columns; logits are e = sum_pos lrelu(u) -
    sum_neg lrelu(-u) via ACT Lrelu with fused accumulation; messages are
    recovered from u via a 1/att columnwise multiply.
  - Gather of xl rows via indirect DMA (gpsimd SWDGE) with CCE add onto an
    xr-broadcast prefill; pads are skipped by the bounds check.
  - Layer-1 output h stays in SBUF, is transposed on PE, and used directly
    as lhsT for the layer-2 GEMMs; xr tables never leave SBUF.
"""
import os
os.environ.setdefault("JAX_PLATFORMS", "cpu")
import sys
if "/opt/trn_rl_repo" not in sys.path:
    sys.path.insert(0, "/opt/trn_rl_repo")
import numpy as np
import concourse.bass as bass
import concourse.bacc as bacc
import concourse.mybir as mybir
import concourse.tile as tile
from concourse import bass_utils
from concourse.masks import make_identity

f32 = mybir.dt.float32
f16 = mybir.dt.float16
i32 = mybir.dt.int32
AX = mybir.AxisListType
OP = mybir.AluOpType
AF = mybir.ActivationFunctionType

N = 50000
NCORES = 8
NP = 50176          # 8 * 6272, multiple of 1024
SH = NP // NCORES   # 6272 = 49 * 128
TPS = SH // 128     # 49 tiles per shard
F_IN = 256
H = 128
C = 64
DMAX = 64
NEG = 0.2
EPS = 1e-16

LAST_RESULT = None
LAST_RUN_WALL = 0.0
_PROGRAM_CACHE = {}


def ts(i, s):
    return slice(i * s, (i + 1) * s)


def ceil4(v):
    return max(4, (int(v) + 3) // 4 * 4)


def edge_phase(nc, tc, Dts, cums, Fp, F, xl_f, xr_sb, slotp, degf_sb, iota_f,
               rc_t, cb_t, relu, out_dram, hT_sb, ident):
    with (
        tc.tile_pool(name=f"pz{F}", bufs=2) as pz,
        tc.tile_pool(name=f"pb{F}", bufs=2) as pb,
        tc.tile_pool(name=f"pm{F}", bufs=3) as psm,
        tc.tile_pool(name=f"ps{F}", bufs=1) as pscr,
        tc.tile_pool(name=f"po{F}", bufs=2) as pout,
        tc.tile_pool(name=f"pp{F}", bufs=2, space="PSUM") as pps,
    ):
        scr = pscr.tile([128, F], f32)
        for t in range(TPS):
            Dt = Dts[t]
            c0 = cums[t]
            idx_t = pb.tile([128, Dt], i32, tag="idx")
            nc.sync.dma_start(out=idx_t[:], in_=slotp.ap()[:, c0:c0 + Dt])
            # pad mask: slot d is a pad iff d >= deg(row) -> -1e30 logit
            off_t = pb.tile([128, Dt], f32, tag="off")
            nc.vector.tensor_scalar(out=off_t[:], in0=iota_f[:, 0:Dt],
                                    scalar1=degf_sb[:, t:t + 1], scalar2=-1e30,
                                    op0=OP.is_ge, op1=OP.mult)

            # z = xr (broadcast prefill) + gathered xl rows (CCE add);
            # pads keep z = xr via the bounds-check skip.
            z_t = pz.tile([128, Dt, F], f32, tag="z")
            nc.gpsimd.tensor_copy(
                out=z_t[:],
                in_=xr_sb[:, t * F:(t + 1) * F][:, None, :]
                .to_broadcast([128, Dt, F]))
            for d in range(Dt):
                nc.gpsimd.indirect_dma_start(
                    out=z_t[:, d, :], out_offset=None, in_=xl_f.ap(),
                    in_offset=bass.IndirectOffsetOnAxis(
                        ap=idx_t[:, d:d + 1], axis=0),
                    bounds_check=NP - 1, oob_is_err=False, compute_op=OP.add)

            # logits: e = sum_pos lrelu(u) - sum_neg lrelu(-u) + off
            ep_t = psm.tile([128, Dt], f32, tag="ep")
            en_t = psm.tile([128, Dt], f32, tag="en")
            for d in range(Dt):
                nc.scalar.activation(out=scr[:, 0:Fp], in_=z_t[:, d, 0:Fp],
                                     func=AF.Prelu, alpha=NEG,
                                     accum_out=ep_t[:, d:d + 1])
                nc.scalar.activation(out=scr[:, 0:F - Fp], in_=z_t[:, d, Fp:F],
                                     func=AF.Prelu, scale=-1.0, alpha=NEG,
                                     accum_out=en_t[:, d:d + 1])
            e_t = psm.tile([128, Dt], f32, tag="e")
            nc.vector.scalar_tensor_tensor(out=e_t[:], in0=en_t[:],
                                           scalar=-1.0, in1=ep_t[:],
                                           op0=OP.mult, op1=OP.add)
            nc.vector.tensor_tensor(out=e_t[:], in0=e_t[:], in1=off_t[:],
                                    op=OP.add)
            mneg_t = psm.tile([128, 1], f32, tag="mneg")
            nc.vector.tensor_reduce(out=mneg_t[:], in_=e_t[:], axis=AX.X,
                                    op=OP.max, negate=True)
            nc.vector.tensor_scalar_min(mneg_t[:], mneg_t[:], 1e29)
            a_t = psm.tile([128, Dt], f32, tag="a")
            nc.scalar.activation(out=a_t[:], in_=e_t[:], func=AF.Exp,
                                 bias=mneg_t[:, :1])
            s_t = psm.tile([128, 1], f32, tag="s")
            nc.vector.tensor_reduce(out=s_t[:], in_=a_t[:], axis=AX.X,
                                    op=OP.add)
            nc.vector.tensor_scalar_add(s_t[:], s_t[:], EPS)
            r_t = psm.tile([128, 1], f32, tag="r")
            nc.vector.reciprocal(out=r_t[:], in_=s_t[:])
            al_t = psm.tile([128, Dt], f32, tag="al")
            nc.vector.tensor_scalar_mul(al_t[:], a_t[:], r_t[:, :1])

            # msg = sum_d alpha_d z_d - (sum alpha) xr  (z holds xr+g)
            acc_t = pout.tile([128, F], f32, tag="acc")
            nc.vector.tensor_scalar(out=acc_t[:], in0=z_t[:, 0, :],
                                    scalar1=al_t[:, 0:1], scalar2=None,
                                    op0=OP.mult)
            for d in range(1, Dt):
                nc.vector.scalar_tensor_tensor(
                    out=acc_t[:], in0=z_t[:, d, :], scalar=al_t[:, d:d + 1],
                    in1=acc_t[:], op0=OP.mult, op1=OP.add)
            saneg_t = psm.tile([128, 1], f32, tag="sa")
            nc.vector.tensor_reduce(out=saneg_t[:], in_=al_t[:],
                                    axis=AX.X, op=OP.add, negate=True)
            hh_t = pout.tile([128, F], f32, tag="hh")
            nc.vector.scalar_tensor_tensor(
                out=hh_t[:], in0=xr_sb[:, t * F:(t + 1) * F],
                scalar=saneg_t[:, :1], in1=acc_t[:], op0=OP.mult, op1=OP.add)
            nc.vector.tensor_tensor(out=hh_t[:], in0=hh_t[:], in1=rc_t[:],
                                    op=OP.mult)
            nc.vector.tensor_tensor(out=hh_t[:], in0=hh_t[:], in1=cb_t[:],
                                    op=OP.add)
            if relu:
                nc.vector.tensor_scalar_max(hh_t[:], hh_t[:], 0.0)
                pt_t = pps.tile([128, 128], f32, tag="pt")
                nc.tensor.transpose(out=pt_t[:], in_=hh_t[:],
                                    identity=ident[:])
                nc.scalar.copy(out=hT_sb[:, ts(t, 128)], in_=pt_t[:])
            else:
                nc.sync.dma_start(out=out_dram.ap()[ts(t, 128), :],
                                  in_=hh_t[:])


def build_program(Dts, Fp1, Fp2):
    key = (tuple(Dts), Fp1, Fp2)
    if key in _PROGRAM_CACHE:
        return _PROGRAM_CACHE[key]
    cums = [0]
    for d in Dts:
        cums.append(cums[-1] + d)
    TOTC = cums[-1]

    nc = bacc.Bacc("TRN2", target_bir_lowering=False, debug=False,
                   enable_asserts=False, num_devices=NCORES)

    xh = nc.dram_tensor("xh", [F_IN, SH], f16, kind="ExternalInput")
    wl1 = nc.dram_tensor("wl1", [F_IN, H], f32, kind="ExternalInput")
    wr1 = nc.dram_tensor("wr1", [F_IN, H], f32, kind="ExternalInput")
    wl2 = nc.dram_tensor("wl2", [H, C], f32, kind="ExternalInput")
    wr2 = nc.dram_tensor("wr2", [H, C], f32, kind="ExternalInput")
    slotp = nc.dram_tensor("slotp", [128, TOTC], i32, kind="ExternalInput")
    degF = nc.dram_tensor("degF", [128, TPS], f32, kind="ExternalInput")
    rc1 = nc.dram_tensor("rc1", [128, H], f32, kind="ExternalInput")
    cb1 = nc.dram_tensor("cb1", [128, H], f32, kind="ExternalInput")
    rc2 = nc.dram_tensor("rc2", [128, C], f32, kind="ExternalInput")
    cb2 = nc.dram_tensor("cb2", [128, C], f32, kind="ExternalInput")
    outc = nc.dram_tensor("outc", [SH, C], f32, kind="ExternalOutput")

    xl1o = nc.dram_tensor("xl1o", [SH, H], f32, kind="Internal")
    xl1f = nc.dram_tensor("xl1f", [NP, H], f32, kind="Internal",
                          addr_space="Shared")
    xl2o = nc.dram_tensor("xl2o", [SH, C], f32, kind="Internal")
    xl2f = nc.dram_tensor("xl2f", [NP, C], f32, kind="Internal",
                          addr_space="Shared")

    with tile.TileContext(nc) as tc:
        with tc.tile_pool(name="persist", bufs=1) as pers:
            xr1_sb = pers.tile([128, TPS * H], f32)
            hT_sb = pers.tile([128, SH], f32)
            xr2_sb = pers.tile([128, TPS * C], f32)
            ident = pers.tile([128, 128], f32)
            make_identity(nc, ident[:])
            iota_i = pers.tile([128, DMAX], i32)
            nc.gpsimd.iota(iota_i[:], [[1, DMAX]], channel_multiplier=0)
            iota_f = pers.tile([128, DMAX], f32)
            nc.scalar.copy(out=iota_f[:], in_=iota_i[:])
            degf_sb = pers.tile([128, TPS], f32)
            nc.sync.dma_start(out=degf_sb[:], in_=degF.ap())
            rc1_t = pers.tile([128, H], f32)
            nc.sync.dma_start(out=rc1_t[:], in_=rc1.ap())
            cb1_t = pers.tile([128, H], f32)
            nc.sync.dma_start(out=cb1_t[:], in_=cb1.ap())
            rc2_t = pers.tile([128, C], f32)
            nc.sync.dma_start(out=rc2_t[:], in_=rc2.ap())
            cb2_t = pers.tile([128, C], f32)
            nc.sync.dma_start(out=cb2_t[:], in_=cb2.ap())

            # ---------------- Phase A: layer-1 GEMMs (own shard) ----------
            with (
                tc.tile_pool(name="paw", bufs=1) as pw,
                tc.tile_pool(name="pa", bufs=4) as pa,
                tc.tile_pool(name="pap", bufs=4, space="PSUM") as pp,
            ):
                wl_t = pw.tile([128, 2, H], f32)
                wr_t = pw.tile([128, 2, H], f32)
                for k in range(2):
                    nc.sync.dma_start(out=wl_t[:, k, :],
                                      in_=wl1.ap()[ts(k, 128), :])
                    nc.sync.dma_start(out=wr_t[:, k, :],
                                      in_=wr1.ap()[ts(k, 128), :])
                for t in range(TPS):
                    xh_t = pa.tile([128, 2, 128], f16, tag="xh")
                    for k in range(2):
                        nc.sync.dma_start(out=xh_t[:, k, :],
                                          in_=xh.ap()[ts(k, 128), ts(t, 128)])
                    xf_t = pa.tile([128, 2, 128], f32, tag="xf")
                    nc.scalar.copy(out=xf_t[:], in_=xh_t[:])
                    psl = pp.tile([128, H], f32, tag="psl")
                    for k in range(2):
                        nc.tensor.matmul(out=psl[:], lhsT=xf_t[:, k, :],
                                         rhs=wl_t[:, k, :],
                                         start=(k == 0), stop=(k == 1))
                    ol = pa.tile([128, H], f32, tag="ol")
                    nc.scalar.copy(out=ol[:], in_=psl[:])
                    nc.sync.dma_start(out=xl1o.ap()[ts(t, 128), :], in_=ol[:])
                    psr = pp.tile([128, H], f32, tag="psr")
                    for k in range(2):
                        nc.tensor.matmul(out=psr[:], lhsT=xf_t[:, k, :],
                                         rhs=wr_t[:, k, :],
                                         start=(k == 0), stop=(k == 1))
                    nc.scalar.copy(out=xr1_sb[:, ts(t, H)], in_=psr[:])

            # ---------------- Phase B: AllGather xl1 ----------------------
            nc.gpsimd.collective_compute(
                "AllGather", OP.bypass,
                replica_groups=[list(range(NCORES))],
                ins=[xl1o.ap()], outs=[xl1f.ap()])

            # ---------------- Phase C: layer-1 edge phase -----------------
            edge_phase(nc, tc, Dts, cums, Fp1, H, xl1f, xr1_sb, slotp,
                       degf_sb, iota_f, rc1_t, cb1_t,
                       relu=True, out_dram=None, hT_sb=hT_sb, ident=ident)

            # ---------------- Phase D: layer-2 GEMMs (from SBUF hT) -------
            with (
                tc.tile_pool(name="pdw", bufs=1) as pw2,
                tc.tile_pool(name="pd", bufs=4) as pd,
                tc.tile_pool(name="pdp", bufs=4, space="PSUM") as pp2,
            ):
                wl2_t = pw2.tile([128, C], f32)
                nc.sync.dma_start(out=wl2_t[:], in_=wl2.ap())
                wr2_t = pw2.tile([128, C], f32)
                nc.sync.dma_start(out=wr2_t[:], in_=wr2.ap())
                for t in range(TPS):
                    ps2 = pp2.tile([128, C], f32, tag="ps2")
                    nc.tensor.matmul(out=ps2[:], lhsT=hT_sb[:, ts(t, 128)],
                                     rhs=wl2_t[:], start=True, stop=True)
                    o2 = pd.tile([128, C], f32, tag="o2")
                    nc.scalar.copy(out=o2[:], in_=ps2[:])
                    nc.sync.dma_start(out=xl2o.ap()[ts(t, 128), :], in_=o2[:])
                    ps3 = pp2.tile([128, C], f32, tag="ps3")
                    nc.tensor.matmul(out=ps3[:], lhsT=hT_sb[:, ts(t, 128)],
                                     rhs=wr2_t[:], start=True, stop=True)
                    nc.scalar.copy(out=xr2_sb[:, ts(t, C)], in_=ps3[:])

            # ---------------- Phase E: AllGather xl2 ----------------------
            nc.gpsimd.collective_compute(
                "AllGather", OP.bypass,
                replica_groups=[list(range(NCORES))],
                ins=[xl2o.ap()], outs=[xl2f.ap()])

            # ---------------- Phase F: layer-2 edge phase -----------------
            edge_phase(nc, tc, Dts, cums, Fp2, C, xl2f, xr2_sb, slotp,
                       degf_sb, iota_f, rc2_t, cb2_t,
                       relu=False, out_dram=outc, hT_sb=None, ident=None)

    nc.compile()
    _PROGRAM_CACHE[key] = nc
    return nc


def prepare_host(x, edge_index, Wl1, Wr1, att1, b1, Wl2, Wr2, att2, b2):
    src = np.asarray(edge_index[0], dtype=np.int64)
    dst = np.asarray(edge_index[1], dtype=np.int64)
    x = np.asarray(x, dtype=np.float32)

    deg = np.bincount(dst, minlength=NP).astype(np.int64)
    assert deg.max() <= DMAX, f"max degree {deg.max()} > {DMAX}"
    order = np.argsort(-deg, kind="stable")
    q = np.arange(NP)
    new_of = np.empty(NP, dtype=np.int64)
    new_of[order] = (q % NCORES) * SH + q // NCORES
    glob_of_new = np.empty(NP, dtype=np.int64)
    glob_of_new[new_of] = np.arange(NP)

    # slot tables (values are NEW ids; rows ordered by NEW id); pads point
    # at row NP so the gather bounds check skips them
    eorder = np.argsort(dst, kind="stable")
    s_src = src[eorder]
    s_dst = dst[eorder]
    starts = np.zeros(NP, dtype=np.int64)
    starts[1:] = np.cumsum(deg)[:-1]
    pos = np.arange(len(s_dst)) - starts[s_dst]
    slot_g = np.full((NP, DMAX), NP, dtype=np.int32)
    slot_g[s_dst, pos] = new_of[s_src].astype(np.int32)
    slot_new = slot_g[glob_of_new]
    deg_new = deg[glob_of_new]

    deg_sorted = deg[order]
    Dts = tuple(ceil4(max(deg_sorted[1024 * t], 1)) for t in range(TPS))

    att1 = np.asarray(att1, np.float32)
    att2 = np.asarray(att2, np.float32)
    assert np.abs(att1).min() > 1e-8 and np.abs(att2).min() > 1e-8
    p1 = np.argsort(att1 < 0, kind="stable")
    Fp1 = int((att1 >= 0).sum())
    p2 = np.argsort(att2 < 0, kind="stable")
    Fp2 = int((att2 >= 0).sum())
    assert 0 < Fp1 < H and 0 < Fp2 < C

    def fold(W, att, perm, rowperm=None):
        Wa = (np.asarray(W, np.float32) * att)
        if rowperm is not None:
            Wa = Wa[rowperm, :]
        return np.ascontiguousarray(Wa[:, perm], np.float32)

    Wl1a = fold(Wl1, att1, p1)
    Wr1a = fold(Wr1, att1, p1)
    Wl2a = fold(Wl2, att2, p2, rowperm=p1)
    Wr2a = fold(Wr2, att2, p2, rowperm=p1)
    rc1_row = (1.0 / att1[p1]).astype(np.float32)
    rc2_row = (1.0 / att2[p2]).astype(np.float32)
    b1_row = np.asarray(b1, np.float32)[p1]
    b2_row = np.asarray(b2, np.float32)[p2]

    xp = np.zeros((NP, F_IN), np.float32)
    xp[:N] = x
    xT_perm = np.ascontiguousarray(xp[glob_of_new].T.astype(np.float16))

    rep = lambda row: np.ascontiguousarray(np.tile(row[None, :], (128, 1)))
    common = dict(
        wl1=Wl1a, wr1=Wr1a, wl2=Wl2a, wr2=Wr2a,
        rc1=rep(rc1_row), cb1=rep(b1_row), rc2=rep(rc2_row), cb2=rep(b2_row))
    in_maps = []
    for c in range(NCORES):
        m = dict(common)
        m["xh"] = np.ascontiguousarray(xT_perm[:, ts(c, SH)])
        sl = slot_new[ts(c, SH)]
        m["slotp"] = np.ascontiguousarray(np.concatenate(
            [sl[ts(t, 128), 0:Dts[t]] for t in range(TPS)], axis=1))
        m["degF"] = np.ascontiguousarray(
            deg_new[ts(c, SH)].reshape(TPS, 128).T.astype(np.float32))
        in_maps.append(m)
    return in_maps, Dts, Fp1, Fp2, glob_of_new, p2


def kernel(**inputs):
    global LAST_RESULT, LAST_RUN_WALL
    import time as _time
    in_maps, Dts, Fp1, Fp2, glob_of_new, p2 = prepare_host(**inputs)
    nc = build_program(Dts, Fp1, Fp2)
    _t0 = _time.time()
    res = bass_utils.run_bass_kernel_spmd(nc, in_maps,
                                          core_ids=list(range(NCORES)))
    LAST_RUN_WALL = _time.time() - _t0
    LAST_RESULT = res
    out_new = np.concatenate([res.results[c]["outc"] for c in range(NCORES)],
                             axis=0)
    out_glob = np.empty((NP, C), np.float32)
    out_glob[glob_of_new] = out_new
    return np.ascontiguousarray(out_glob[:N][:, np.argsort(p2)])
